# revision 26
# baseline (speedup 1.0000x reference)
"""Trainium2 Bass kernel for quantized DenseGeneral (AQT-style int8 fake-quant matmul).

Math (must match the jax reference):
  xq = round(x / sx) * sx    sx = max(amax_row(|x|), 1e-6)/127     (per-row of x)
  kq = round(k / sk) * sk    sk = max(amax_col(|k|), 1e-6)/127     (per-col of k)
  out = xq @ kq  =  (sx (x) sk) * (x_int @ k_int)

x_int/k_int are integers in [-127, 127] — exactly representable in bf16, so the
matmul runs on the PE array in bf16 with fp32 PSUM accumulation and is exact.
round() uses the fp32 magic-number trick (t + 1.5*2^23 - 1.5*2^23), matching
jnp.round's round-half-to-even.

Sharding over 8 cores: 4-way on flattened rows, 2-way on output columns F.
Per core: x [2048, 4096], k [4096, 2048] -> out [2048, 2048].

v7 — single k read with fp16 stash + engine rebalance + big DMAs:
  - pass-1 streams k once (32 x 1MB on the ACT HWDGE ring). Per chunk: ACT
    casts the f32 chunk to fp16 and stashes it into the resident kq buffer
    (bitcast view), ACT takes |kt| in place, DVE max-accumulates the exact
    f32 amax into kacc. fp16 (10 mantissa bits) makes round(fp16(k)*kinv)
    vs round(k*kinv) flips rare: ~1.4e-3 output rel err (gate is 2e-2).
  - finalize: Pool partition_all_reduce -> amax; sk = amax/127 (ACT, bf16);
    kinv = 127/amax (DVE reciprocal; ACT reciprocal is banned for accuracy).
  - pass-2 is DMA-free: Pool multiplies the fp16 stash by kinv into an f32
    staging tile (walrus accepts Pool mult with full-tile output; max/stt on
    Pool are rejected), DVE magic-rounds it back into kq as bf16 ints. kq
    production is engine-paced, so the PE ramps right after finalize.
  - x quant: SP-ring loads (1MB halves), DVE abs-max reduces; the magic round
    runs as op1 (t = x*inv + MAGIC: DVE for tiles 0-3 which land during
    pass-1 while ACT streams k; ACT Copy(scale=inv, bias=MAGIC) for tiles
    4-15) and op2 (t - MAGIC -> bf16 ints, DVE, quarter tiles). xq is stored
    to DRAM in an mt-blocked [mt, dc, 128m, 128f] layout; ONE contiguous 1MB
    XBAR transpose-read per mt yields lhsT [128d, dc, 128m].
  - DVE max-accs are emitted interleaved with x tiles 0-3 so the 2-slot kt
    staging ring keeps releasing and the k DMA stream never stalls on DVE.
  - epilogue: one DVE scalar_tensor_tensor per [128, 512] PSUM tile
    (osb = (psum*sx)*sk, sk in bf16), out DMA on the ACT ring so the SP ring
    (x loads / xq writes / transposes) never queues behind output stores.
  - DMA instruction count per iteration: ~210 (vs ~700 in v6) — HWDGE issue
    serialization (~0.6us each) was a main v6 bottleneck.

The body is fully self-contained (no prologue): single-shot N=1 runs the same
instruction stream as each loop iteration, and the body is idempotent (kacc
memset + identical recompute each iteration), so the N-delta benchmark measures
the same program the harness runs once.
"""

import os
import sys
from contextlib import ExitStack

import numpy as np

if "/opt/trn_rl_repo" not in sys.path and os.path.isdir("/opt/trn_rl_repo"):
    sys.path.insert(0, "/opt/trn_rl_repo")

import concourse.bass as bass
import concourse.mybir as mybir
import concourse.tile as tile
from concourse import bacc, bass_isa

# Problem geometry (hardcoded per contract)
B, S, DIM, F_FULL = 4, 2048, 4096, 4096
M_FULL = B * S              # 8192 flattened rows
N_CORES = 8
M_SHARDS, F_SHARDS = 4, 2   # core c -> (mi, fi) = divmod(c, F_SHARDS)
M = M_FULL // M_SHARDS      # 2048 rows per core
F = F_FULL // F_SHARDS      # 2048 output cols per core
P = 128
DCH = DIM // P              # 32 contraction chunks
MT = M // P                 # 16 row tiles
FS = 512                    # matmul free dim (one PSUM bank of fp32)
FT = F // FS                # 4 output column strips
NSB = MT // 2               # 8 superblocks of 2 row tiles

MAGIC = float(np.float32(1.5 * 2**23))  # 12582912.0
INT8_MAX = 127.0

f32 = mybir.dt.float32
bf16 = mybir.dt.bfloat16
f16 = mybir.dt.float16
Alu = mybir.AluOpType
Act = mybir.ActivationFunctionType


def build_bass(niter: int = 1):
    """niter > 1 wraps the body in a hardware For loop — used only for
    benchmarking (kernel time = delta(wall) / delta(niter) cancels host I/O).
    The body is idempotent so the loop repeats the exact single-shot work."""
    nc = bacc.Bacc("TRN2", target_bir_lowering=False, enable_partition_id=False)

    x_in = nc.dram_tensor("x", [M, DIM], f32, kind="ExternalInput")
    k_in = nc.dram_tensor("kern", [DIM, F], f32, kind="ExternalInput")
    out = nc.dram_tensor("out", [M, F], f32, kind="ExternalOutput")

    with tile.TileContext(nc) as tc, ExitStack() as ctx:
        dram = ctx.enter_context(tc.tile_pool(name="dram", bufs=1, space="DRAM"))
        # quantized x ints, mt-blocked layout [mt, dc, m, 128f]: the whole mt
        # block [dc*m, 128] is contiguous, so ONE 1MB XBAR transpose read per
        # mt produces lhsT [128d, dc, m] (strided sub-2KB sources are slow).
        xq_dram = dram.tile([MT, DCH, P, P], bf16)

        persist = ctx.enter_context(tc.tile_pool(name="persist", bufs=1))
        # resident quantized kernel: 32 full tiles (walrus rejects sliced Pool
        # outputs, so each chunk is its own full-tile tensor). Doubles as the
        # fp16 stash between pass-1 and pass-2 (bitcast view; same 2B/elem).
        kqt = [persist.tile([P, F], bf16, tag=f"kq{dc}", name=f"kq{dc}")
               for dc in range(DCH)]
        kinv = persist.tile([P, F], f32)        # amax -> 127/amax (quant scale)
        sk = persist.tile([P, F], bf16)         # amax/127 (epilogue scale)
        kmax = persist.tile([P, F], f16)        # pass-1 stash max accumulator
        kmin = persist.tile([P, F], f16)        # pass-1 stash min accumulator
        sx_all = persist.tile([P, MT], f32)     # per-row scales, col mt

        xhp = ctx.enter_context(tc.tile_pool(name="xh", bufs=2))
        xsp = ctx.enter_context(tc.tile_pool(name="xs", bufs=4))
        xqp = ctx.enter_context(tc.tile_pool(name="xqo", bufs=2))
        ktp = ctx.enter_context(tc.tile_pool(name="kt", bufs=2))
        xtp = ctx.enter_context(tc.tile_pool(name="xt", bufs=2))
        psp = ctx.enter_context(tc.tile_pool(name="ps", bufs=8, space="PSUM"))
        osp = ctx.enter_context(tc.tile_pool(name="osb", bufs=2))

        def emit_pass1_k(dc):
            """Stream one k chunk: ACT dma + fp16 stash cast. The staging slot
            frees right after the cast (2-deep ring tracks the DMA stream);
            amax accumulation runs decoupled on the stash."""
            kt = ktp.tile([P, F], f32, tag="kt", name=f"p1_{dc}")
            nc.gpsimd.dma_start(kt[:], k_in[dc * P:(dc + 1) * P, :])
            nc.scalar.activation(kqt[dc][:].bitcast(f16), kt[:], Act.Copy)

        def emit_accum(dc):
            """max/min-accumulate the fp16 stash (2-byte DVE fast path).
            amax derived from the stash is off by <=2^-11 relative — the
            resulting extra rounding flips stay well inside the 2e-2 gate."""
            nc.vector.tensor_tensor(kmax[:], kqt[dc][:].bitcast(f16), kmax[:],
                                    Alu.max)
            nc.vector.tensor_tensor(kmin[:], kqt[dc][:].bitcast(f16), kmin[:],
                                    Alu.min)

        def emit_finalize():
            """amax = max(kmax, -kmin) -> f32; kinv = 127/amax; sk = amax/127."""
            u16 = mybir.dt.uint16
            nc.vector.tensor_scalar(kmin[:].bitcast(u16), kmin[:].bitcast(u16),
                                    0x8000, None, Alu.bitwise_xor)
            nc.vector.tensor_tensor(kmax[:], kmin[:], kmax[:], Alu.max)
            amaxf = ktp.tile([P, F], f32, tag="kt", name="amaxf")
            nc.vector.tensor_scalar(amaxf[:], kmax[:], 0.0, None, Alu.add)
            nc.gpsimd.partition_all_reduce(kinv[:], amaxf[:], P,
                                           bass_isa.ReduceOp.max)
            nc.vector.tensor_scalar_max(kinv[:], kinv[:], 1e-6)
            nc.scalar.activation(sk[:], kinv[:], Act.Copy, scale=1.0 / INT8_MAX)
            nc.vector.reciprocal(kinv[:], kinv[:])
            nc.vector.tensor_scalar_mul(kinv[:], kinv[:], INT8_MAX)

        def emit_pass2():
            """DMA-free quantize: kq = round(stash_fp16 * kinv) as bf16 ints.
            Mults alternate Pool/DVE so kq production outpaces mm consumption
            (Pool-only mults at 4us/chunk starved the PE for ~100us)."""
            for dc in range(DCH):
                kt2 = ktp.tile([P, F], f32, tag="kt", name=f"p2_{dc}")
                eng = nc.gpsimd if dc % 2 == 0 else nc.vector
                eng.tensor_tensor(kt2[:], kqt[dc][:].bitcast(f16),
                                  kinv[:], Alu.mult)
                if dc % 2 == 0:
                    nc.vector.tensor_scalar(kqt[dc][:], kt2[:], MAGIC, -MAGIC,
                                            Alu.add, Alu.add)
                else:
                    # ACT round: in-place +MAGIC (f32), then -MAGIC -> bf16
                    nc.scalar.activation(kt2[:], kt2[:], Act.Copy, bias=MAGIC)
                    nc.scalar.activation(kqt[dc][:], kt2[:], Act.Copy,
                                         bias=-MAGIC)

        def emit_xquant(mt, act_round):
            rows = slice(mt * P, (mt + 1) * P)
            xh0 = xhp.tile([P, DIM // 2], f32, tag="xh", name=f"xh0_{mt}")
            xh1 = xhp.tile([P, DIM // 2], f32, tag="xh", name=f"xh1_{mt}")
            nc.sync.dma_start(xh0[:], x_in[rows, :DIM // 2])
            nc.sync.dma_start(xh1[:], x_in[rows, DIM // 2:])
            a0 = xsp.tile([P, 1], f32, tag="ax", name=f"a0_{mt}")
            a1 = xsp.tile([P, 1], f32, tag="ax", name=f"a1_{mt}")
            nc.vector.tensor_reduce(a0[:], xh0[:], axis=mybir.AxisListType.X,
                                    op=Alu.max, apply_absolute_value=True)
            nc.vector.tensor_reduce(a1[:], xh1[:], axis=mybir.AxisListType.X,
                                    op=Alu.max, apply_absolute_value=True)
            ax = xsp.tile([P, 1], f32, tag="ax", name=f"ax_{mt}")
            nc.vector.tensor_tensor(ax[:], a0[:], a1[:], Alu.max)
            # sx = max(ax, 1e-6)/127 in one op; inv = 1/sx = 127/amax
            nc.vector.tensor_scalar(sx_all[:, mt:mt + 1], ax[:], 1e-6,
                                    1.0 / INT8_MAX, Alu.max, Alu.mult)
            inv = xsp.tile([P, 1], f32, tag="ax", name=f"inv_{mt}")
            nc.vector.reciprocal(inv[:], sx_all[:, mt:mt + 1])
            for h, xh in ((0, xh0), (1, xh1)):
                # op1: t = x*inv + MAGIC (fp32, in place). ACT during the mm
                # phase (idle there), Pool during pass-1 (idle there; ACT is
                # busy with k casts and DVE with amax accumulation).
                if act_round:
                    nc.scalar.activation(xh[:], xh[:], Act.Copy,
                                         scale=inv[:, :1], bias=MAGIC)
                else:
                    nc.gpsimd.tensor_scalar(xh[:], xh[:], inv[:, :1], MAGIC,
                                            Alu.mult, Alu.add)
                # op2: quarters, t - MAGIC -> bf16 ints, then store
                for q in range(2):
                    qcols = slice(q * (DIM // 4), (q + 1) * (DIM // 4))
                    xqo = xqp.tile([P, DIM // 4], bf16, tag="xqo",
                                   name=f"xqo{h}{q}_{mt}")
                    nc.vector.tensor_scalar(xqo[:], xh[:, qcols], -MAGIC, None,
                                            Alu.add)
                    dc0 = h * (DCH // 2) + q * (DCH // 4)
                    nc.sync.dma_start(
                        xq_dram[mt, dc0:dc0 + DCH // 4]
                        .rearrange("dc m f -> m dc f"),
                        xqo[:].rearrange("m (dc f) -> m dc f", f=P))

        def emit_transpose(mt):
            xt = xtp.tile([P, DCH, P], bf16, tag="xqT", name=f"xt_{mt}")
            nc.sync.dma_start_transpose(
                xt[:], xq_dram[mt].rearrange("dc m f -> (dc m) f"))
            return xt

        def emit_superblock(sb, xts):
            for ml in range(2):
                mt = 2 * sb + ml
                mrows = slice(mt * P, (mt + 1) * P)
                psums = [psp.tile([P, FS], f32, tag="ps", name=f"ps{mt}_{i}")
                         for i in range(FT)]
                for dc in range(DCH):
                    lhsT = xts[ml][:, dc, :]
                    for fs in range(FT):
                        nc.tensor.matmul(psums[fs][:], lhsT,
                                         kqt[dc][:, fs * FS:(fs + 1) * FS],
                                         start=(dc == 0), stop=(dc == DCH - 1))
                for fs in range(FT):
                    osb = osp.tile([P, FS], f32, tag="osb",
                                   name=f"osb{mt}_{fs}")
                    # fused epilogue: osb = (psum * sx) * sk in one DVE op
                    nc.vector.scalar_tensor_tensor(
                        osb[:], psums[fs][:], sx_all[:, mt:mt + 1],
                        sk[:, fs * FS:(fs + 1) * FS],
                        Alu.mult, Alu.mult)
                    nc.sync.dma_start(out[mrows, fs * FS:(fs + 1) * FS],
                                       osb[:])

        def emit_body():
            nc.vector.memset(kmax[:], float("-inf"))
            nc.vector.memset(kmin[:], float("inf"))
            # pass-1 k DMA+cast stream first (ring drains at DMA pace); only
            # x tiles 0-1 run during it (sb0 needs them) so k gets the DMA
            # bandwidth; the stash accumulation is decoupled on DVE.
            for dc in range(DCH):
                emit_pass1_k(dc)
            emit_xquant(0, act_round=False)
            emit_xquant(1, act_round=False)
            for dc in range(DCH):
                emit_accum(dc)
            emit_finalize()
            emit_pass2()
            xts = [emit_transpose(0), emit_transpose(1)]
            for sb in range(NSB):
                nxts = None
                if sb < NSB - 1:
                    emit_xquant(2 * sb + 2, act_round=(sb > 0))
                    emit_xquant(2 * sb + 3, act_round=(sb > 0))
                    nxts = [emit_transpose(2 * sb + 2),
                            emit_transpose(2 * sb + 3)]
                emit_superblock(sb, xts)
                xts = nxts

        if niter > 1:
            with tc.For_i(0, niter, 1):
                emit_body()
        else:
            emit_body()

    nc.compile()
    return nc


_NC_CACHE = None


def _get_nc():
    global _NC_CACHE
    if _NC_CACHE is None:
        _NC_CACHE = build_bass()
    return _NC_CACHE


def make_in_maps(inputs: np.ndarray, kernel: np.ndarray):
    x = np.ascontiguousarray(np.asarray(inputs, np.float32).reshape(M_FULL, DIM))
    w = np.asarray(kernel, np.float32)
    in_maps = []
    for c in range(N_CORES):
        mi, fi = divmod(c, F_SHARDS)
        in_maps.append({
            "x": np.ascontiguousarray(x[mi * M:(mi + 1) * M]),
            "kern": np.ascontiguousarray(w[:, fi * F:(fi + 1) * F]),
        })
    return in_maps


def assemble_out(shards):
    out = np.empty((M_FULL, F_FULL), np.float32)
    for c in range(N_CORES):
        mi, fi = divmod(c, F_SHARDS)
        out[mi * M:(mi + 1) * M, fi * F:(fi + 1) * F] = shards[c]
    return out.reshape(B, S, F_FULL)


def kernel(inputs: np.ndarray, kernel: np.ndarray, _trace: bool = False):
    from concourse.bass_utils import run_bass_kernel_spmd

    nc = _get_nc()
    res = run_bass_kernel_spmd(nc, make_in_maps(inputs, kernel),
                               core_ids=list(range(N_CORES)), trace=_trace)
    out = assemble_out([r["out"] for r in res.results])
    if _trace:
        return out, res
    return out


# revision 27
# speedup vs baseline: 1.0529x; 1.0529x over previous
"""Trainium2 Bass kernel for quantized DenseGeneral (AQT-style int8 fake-quant matmul).

Math (must match the jax reference):
  xq = round(x / sx) * sx    sx = max(amax_row(|x|), 1e-6)/127     (per-row of x)
  kq = round(k / sk) * sk    sk = max(amax_col(|k|), 1e-6)/127     (per-col of k)
  out = xq @ kq  =  (sx (x) sk) * (x_int @ k_int)

x_int/k_int are integers in [-127, 127] — exactly representable in bf16, so the
matmul runs on the PE array in bf16 with fp32 PSUM accumulation and is exact.
round() uses the fp32 magic-number trick (t + 1.5*2^23 - 1.5*2^23), matching
jnp.round's round-half-to-even.

Sharding over 8 cores: 4-way on flattened rows, 2-way on output columns F.
Per core: x [2048, 4096], k [4096, 2048] -> out [2048, 2048].

v7 — single k read with fp16 stash + engine rebalance + big DMAs:
  - pass-1 streams k once (32 x 1MB on the ACT HWDGE ring). Per chunk: ACT
    casts the f32 chunk to fp16 and stashes it into the resident kq buffer
    (bitcast view), ACT takes |kt| in place, DVE max-accumulates the exact
    f32 amax into kacc. fp16 (10 mantissa bits) makes round(fp16(k)*kinv)
    vs round(k*kinv) flips rare: ~1.4e-3 output rel err (gate is 2e-2).
  - finalize: Pool partition_all_reduce -> amax; sk = amax/127 (ACT, bf16);
    kinv = 127/amax (DVE reciprocal; ACT reciprocal is banned for accuracy).
  - pass-2 is DMA-free: Pool multiplies the fp16 stash by kinv into an f32
    staging tile (walrus accepts Pool mult with full-tile output; max/stt on
    Pool are rejected), DVE magic-rounds it back into kq as bf16 ints. kq
    production is engine-paced, so the PE ramps right after finalize.
  - x quant: SP-ring loads (1MB halves), DVE abs-max reduces; the magic round
    runs as op1 (t = x*inv + MAGIC: DVE for tiles 0-3 which land during
    pass-1 while ACT streams k; ACT Copy(scale=inv, bias=MAGIC) for tiles
    4-15) and op2 (t - MAGIC -> bf16 ints, DVE, quarter tiles). xq is stored
    to DRAM in an mt-blocked [mt, dc, 128m, 128f] layout; ONE contiguous 1MB
    XBAR transpose-read per mt yields lhsT [128d, dc, 128m].
  - DVE max-accs are emitted interleaved with x tiles 0-3 so the 2-slot kt
    staging ring keeps releasing and the k DMA stream never stalls on DVE.
  - epilogue: one DVE scalar_tensor_tensor per [128, 512] PSUM tile
    (osb = (psum*sx)*sk, sk in bf16), out DMA on the ACT ring so the SP ring
    (x loads / xq writes / transposes) never queues behind output stores.
  - DMA instruction count per iteration: ~210 (vs ~700 in v6) — HWDGE issue
    serialization (~0.6us each) was a main v6 bottleneck.

The body is fully self-contained (no prologue): single-shot N=1 runs the same
instruction stream as each loop iteration, and the body is idempotent (kacc
memset + identical recompute each iteration), so the N-delta benchmark measures
the same program the harness runs once.
"""

import os
import sys
from contextlib import ExitStack

import numpy as np

if "/opt/trn_rl_repo" not in sys.path and os.path.isdir("/opt/trn_rl_repo"):
    sys.path.insert(0, "/opt/trn_rl_repo")

import concourse.bass as bass
import concourse.mybir as mybir
import concourse.tile as tile
from concourse import bacc, bass_isa

# Problem geometry (hardcoded per contract)
B, S, DIM, F_FULL = 4, 2048, 4096, 4096
M_FULL = B * S              # 8192 flattened rows
N_CORES = 8
M_SHARDS, F_SHARDS = 4, 2   # core c -> (mi, fi) = divmod(c, F_SHARDS)
M = M_FULL // M_SHARDS      # 2048 rows per core
F = F_FULL // F_SHARDS      # 2048 output cols per core
P = 128
DCH = DIM // P              # 32 contraction chunks
MT = M // P                 # 16 row tiles
FS = 512                    # matmul free dim (one PSUM bank of fp32)
FT = F // FS                # 4 output column strips
NSB = MT // 2               # 8 superblocks of 2 row tiles

MAGIC = float(np.float32(1.5 * 2**23))  # 12582912.0
INT8_MAX = 127.0

f32 = mybir.dt.float32
bf16 = mybir.dt.bfloat16
f16 = mybir.dt.float16
Alu = mybir.AluOpType
Act = mybir.ActivationFunctionType


def build_bass(niter: int = 1):
    """niter > 1 wraps the body in a hardware For loop — used only for
    benchmarking (kernel time = delta(wall) / delta(niter) cancels host I/O).
    The body is idempotent so the loop repeats the exact single-shot work."""
    nc = bacc.Bacc("TRN2", target_bir_lowering=False, enable_partition_id=False)

    x_in = nc.dram_tensor("x", [M, DIM], f32, kind="ExternalInput")
    k_in = nc.dram_tensor("kern", [DIM, F], f32, kind="ExternalInput")
    out = nc.dram_tensor("out", [M, F], f32, kind="ExternalOutput")

    with tile.TileContext(nc) as tc, ExitStack() as ctx:
        dram = ctx.enter_context(tc.tile_pool(name="dram", bufs=1, space="DRAM"))
        # quantized x ints, mt-blocked layout [mt, dc, m, 128f]: the whole mt
        # block [dc*m, 128] is contiguous, so ONE 1MB XBAR transpose read per
        # mt produces lhsT [128d, dc, m] (strided sub-2KB sources are slow).
        xq_dram = dram.tile([MT, DCH, P, P], bf16)

        persist = ctx.enter_context(tc.tile_pool(name="persist", bufs=1))
        # resident quantized kernel: 32 full tiles (walrus rejects sliced Pool
        # outputs, so each chunk is its own full-tile tensor). Doubles as the
        # fp16 stash between pass-1 and pass-2 (bitcast view; same 2B/elem).
        kqt = [persist.tile([P, F], bf16, tag=f"kq{dc}", name=f"kq{dc}")
               for dc in range(DCH)]
        kinv = persist.tile([P, F], f32)        # amax -> 127/amax (quant scale)
        sk = persist.tile([P, F], bf16)         # amax/127 (epilogue scale)
        kmax = persist.tile([P, F], f16)        # pass-1 stash max accumulator
        kmin = persist.tile([P, F], f16)        # pass-1 stash min accumulator
        sx_all = persist.tile([P, MT], f32)     # per-row scales, col mt

        xhp = ctx.enter_context(tc.tile_pool(name="xh", bufs=2))
        xsp = ctx.enter_context(tc.tile_pool(name="xs", bufs=4))
        xqp = ctx.enter_context(tc.tile_pool(name="xqo", bufs=2))
        ktp = ctx.enter_context(tc.tile_pool(name="kt", bufs=2))
        xtp = ctx.enter_context(tc.tile_pool(name="xt", bufs=2))
        psp = ctx.enter_context(tc.tile_pool(name="ps", bufs=8, space="PSUM"))
        osp = ctx.enter_context(tc.tile_pool(name="osb", bufs=2))

        def emit_pass1_k(dc):
            """Stream one k chunk: ACT dma + fp16 stash cast. The staging slot
            frees right after the cast (2-deep ring tracks the DMA stream);
            amax accumulation runs decoupled on the stash."""
            kt = ktp.tile([P, F], f32, tag="kt", name=f"p1_{dc}")
            nc.scalar.dma_start(kt[:], k_in[dc * P:(dc + 1) * P, :])
            nc.scalar.activation(kqt[dc][:].bitcast(f16), kt[:], Act.Copy)

        def emit_accum(dc):
            """max/min-accumulate the fp16 stash (2-byte DVE fast path).
            amax derived from the stash is off by <=2^-11 relative — the
            resulting extra rounding flips stay well inside the 2e-2 gate."""
            nc.vector.tensor_tensor(kmax[:], kqt[dc][:].bitcast(f16), kmax[:],
                                    Alu.max)
            nc.vector.tensor_tensor(kmin[:], kqt[dc][:].bitcast(f16), kmin[:],
                                    Alu.min)

        def emit_finalize():
            """amax = max(kmax, -kmin) -> f32; kinv = 127/amax; sk = amax/127."""
            u16 = mybir.dt.uint16
            nc.vector.tensor_scalar(kmin[:].bitcast(u16), kmin[:].bitcast(u16),
                                    0x8000, None, Alu.bitwise_xor)
            nc.vector.tensor_tensor(kmax[:], kmin[:], kmax[:], Alu.max)
            amaxf = ktp.tile([P, F], f32, tag="kt", name="amaxf")
            nc.vector.tensor_scalar(amaxf[:], kmax[:], 0.0, None, Alu.add)
            nc.gpsimd.partition_all_reduce(kinv[:], amaxf[:], P,
                                           bass_isa.ReduceOp.max)
            nc.vector.tensor_scalar_max(kinv[:], kinv[:], 1e-6)
            nc.scalar.activation(sk[:], kinv[:], Act.Copy, scale=1.0 / INT8_MAX)
            nc.vector.reciprocal(kinv[:], kinv[:])
            nc.vector.tensor_scalar_mul(kinv[:], kinv[:], INT8_MAX)

        def emit_pass2():
            """DMA-free quantize: kq = round(stash_fp16 * kinv) as bf16 ints.
            Mults alternate Pool/DVE so kq production outpaces mm consumption
            (Pool-only mults at 4us/chunk starved the PE for ~100us)."""
            for dc in range(DCH):
                kt2 = ktp.tile([P, F], f32, tag="kt", name=f"p2_{dc}")
                eng = nc.gpsimd if dc % 2 == 0 else nc.vector
                eng.tensor_tensor(kt2[:], kqt[dc][:].bitcast(f16),
                                  kinv[:], Alu.mult)
                if dc % 2 == 0:
                    nc.vector.tensor_scalar(kqt[dc][:], kt2[:], MAGIC, -MAGIC,
                                            Alu.add, Alu.add)
                else:
                    # ACT round: in-place +MAGIC (f32), then -MAGIC -> bf16
                    nc.scalar.activation(kt2[:], kt2[:], Act.Copy, bias=MAGIC)
                    nc.scalar.activation(kqt[dc][:], kt2[:], Act.Copy,
                                         bias=-MAGIC)

        def emit_xquant(mt, act_round):
            rows = slice(mt * P, (mt + 1) * P)
            xh0 = xhp.tile([P, DIM // 2], f32, tag="xh", name=f"xh0_{mt}")
            xh1 = xhp.tile([P, DIM // 2], f32, tag="xh", name=f"xh1_{mt}")
            nc.sync.dma_start(xh0[:], x_in[rows, :DIM // 2])
            nc.sync.dma_start(xh1[:], x_in[rows, DIM // 2:])
            a0 = xsp.tile([P, 1], f32, tag="ax", name=f"a0_{mt}")
            a1 = xsp.tile([P, 1], f32, tag="ax", name=f"a1_{mt}")
            nc.vector.tensor_reduce(a0[:], xh0[:], axis=mybir.AxisListType.X,
                                    op=Alu.max, apply_absolute_value=True)
            nc.vector.tensor_reduce(a1[:], xh1[:], axis=mybir.AxisListType.X,
                                    op=Alu.max, apply_absolute_value=True)
            ax = xsp.tile([P, 1], f32, tag="ax", name=f"ax_{mt}")
            nc.vector.tensor_tensor(ax[:], a0[:], a1[:], Alu.max)
            # sx = max(ax, 1e-6)/127 in one op; inv = 1/sx = 127/amax
            nc.vector.tensor_scalar(sx_all[:, mt:mt + 1], ax[:], 1e-6,
                                    1.0 / INT8_MAX, Alu.max, Alu.mult)
            inv = xsp.tile([P, 1], f32, tag="ax", name=f"inv_{mt}")
            nc.vector.reciprocal(inv[:], sx_all[:, mt:mt + 1])
            for h, xh in ((0, xh0), (1, xh1)):
                # op1: t = x*inv + MAGIC (fp32, in place). ACT during the mm
                # phase (idle there), Pool during pass-1 (idle there; ACT is
                # busy with k casts and DVE with amax accumulation).
                if act_round:
                    nc.scalar.activation(xh[:], xh[:], Act.Copy,
                                         scale=inv[:, :1], bias=MAGIC)
                else:
                    nc.gpsimd.tensor_scalar(xh[:], xh[:], inv[:, :1], MAGIC,
                                            Alu.mult, Alu.add)
                # op2: quarters, t - MAGIC -> bf16 ints, then store
                for q in range(2):
                    qcols = slice(q * (DIM // 4), (q + 1) * (DIM // 4))
                    xqo = xqp.tile([P, DIM // 4], bf16, tag="xqo",
                                   name=f"xqo{h}{q}_{mt}")
                    nc.vector.tensor_scalar(xqo[:], xh[:, qcols], -MAGIC, None,
                                            Alu.add)
                    dc0 = h * (DCH // 2) + q * (DCH // 4)
                    nc.sync.dma_start(
                        xq_dram[mt, dc0:dc0 + DCH // 4]
                        .rearrange("dc m f -> m dc f"),
                        xqo[:].rearrange("m (dc f) -> m dc f", f=P))

        def emit_transpose(mt):
            xt = xtp.tile([P, DCH, P], bf16, tag="xqT", name=f"xt_{mt}")
            nc.sync.dma_start_transpose(
                xt[:], xq_dram[mt].rearrange("dc m f -> (dc m) f"))
            return xt

        def emit_superblock(sb, xts):
            for ml in range(2):
                mt = 2 * sb + ml
                mrows = slice(mt * P, (mt + 1) * P)
                psums = [psp.tile([P, FS], f32, tag="ps", name=f"ps{mt}_{i}")
                         for i in range(FT)]
                for dc in range(DCH):
                    lhsT = xts[ml][:, dc, :]
                    for fs in range(FT):
                        nc.tensor.matmul(psums[fs][:], lhsT,
                                         kqt[dc][:, fs * FS:(fs + 1) * FS],
                                         start=(dc == 0), stop=(dc == DCH - 1))
                for fs in range(FT):
                    osb = osp.tile([P, FS], f32, tag="osb",
                                   name=f"osb{mt}_{fs}")
                    # fused epilogue: osb = (psum * sx) * sk in one DVE op
                    nc.vector.scalar_tensor_tensor(
                        osb[:], psums[fs][:], sx_all[:, mt:mt + 1],
                        sk[:, fs * FS:(fs + 1) * FS],
                        Alu.mult, Alu.mult)
                    nc.scalar.dma_start(out[mrows, fs * FS:(fs + 1) * FS],
                                        osb[:])

        def emit_body():
            nc.vector.memset(kmax[:], float("-inf"))
            nc.vector.memset(kmin[:], float("inf"))
            # pass-1 k DMA+cast stream first (ring drains at DMA pace), then
            # x tiles 0-3 interleaved with the decoupled stash accumulation.
            for dc in range(DCH):
                emit_pass1_k(dc)
            for g in range(4):
                emit_xquant(g, act_round=False)
                for dc in range(g * 8, (g + 1) * 8):
                    emit_accum(dc)
            emit_finalize()
            emit_pass2()
            xts = [emit_transpose(0), emit_transpose(1)]
            for sb in range(NSB):
                nxts = None
                if sb < NSB - 1:
                    if 2 * sb + 4 < MT:
                        emit_xquant(2 * sb + 4, act_round=True)
                    if 2 * sb + 5 < MT:
                        emit_xquant(2 * sb + 5, act_round=True)
                    nxts = [emit_transpose(2 * sb + 2),
                            emit_transpose(2 * sb + 3)]
                emit_superblock(sb, xts)
                xts = nxts

        if niter > 1:
            with tc.For_i(0, niter, 1):
                emit_body()
        else:
            emit_body()

    nc.compile()
    return nc


_NC_CACHE = None


def _get_nc():
    global _NC_CACHE
    if _NC_CACHE is None:
        _NC_CACHE = build_bass()
    return _NC_CACHE


def make_in_maps(inputs: np.ndarray, kernel: np.ndarray):
    x = np.ascontiguousarray(np.asarray(inputs, np.float32).reshape(M_FULL, DIM))
    w = np.asarray(kernel, np.float32)
    in_maps = []
    for c in range(N_CORES):
        mi, fi = divmod(c, F_SHARDS)
        in_maps.append({
            "x": np.ascontiguousarray(x[mi * M:(mi + 1) * M]),
            "kern": np.ascontiguousarray(w[:, fi * F:(fi + 1) * F]),
        })
    return in_maps


def assemble_out(shards):
    out = np.empty((M_FULL, F_FULL), np.float32)
    for c in range(N_CORES):
        mi, fi = divmod(c, F_SHARDS)
        out[mi * M:(mi + 1) * M, fi * F:(fi + 1) * F] = shards[c]
    return out.reshape(B, S, F_FULL)


def kernel(inputs: np.ndarray, kernel: np.ndarray, _trace: bool = False):
    from concourse.bass_utils import run_bass_kernel_spmd

    nc = _get_nc()
    res = run_bass_kernel_spmd(nc, make_in_maps(inputs, kernel),
                               core_ids=list(range(N_CORES)), trace=_trace)
    out = assemble_out([r["out"] for r in res.results])
    if _trace:
        return out, res
    return out


# revision 30
# speedup vs baseline: 1.1243x; 1.0679x over previous
"""Trainium2 Bass kernel for quantized DenseGeneral (AQT-style int8 fake-quant matmul).

Math (must match the jax reference):
  xq = round(x / sx) * sx    sx = max(amax_row(|x|), 1e-6)/127     (per-row of x)
  kq = round(k / sk) * sk    sk = max(amax_col(|k|), 1e-6)/127     (per-col of k)
  out = xq @ kq  =  (sx (x) sk) * (x_int @ k_int)

x_int/k_int are integers in [-127, 127] — exactly representable in bf16, so the
matmul runs on the PE array in bf16 with fp32 PSUM accumulation and is exact.
round() uses the fp32 magic-number trick (t + 1.5*2^23 - 1.5*2^23), matching
jnp.round's round-half-to-even.

Sharding over 8 cores: 4-way on flattened rows, 2-way on output columns F.
Per core: x [2048, 4096], k [4096, 2048] -> out [2048, 2048].

v7 — single k read with fp16 stash + engine rebalance + big DMAs:
  - pass-1 streams k once (32 x 1MB on the ACT HWDGE ring). Per chunk: ACT
    casts the f32 chunk to fp16 and stashes it into the resident kq buffer
    (bitcast view), ACT takes |kt| in place, DVE max-accumulates the exact
    f32 amax into kacc. fp16 (10 mantissa bits) makes round(fp16(k)*kinv)
    vs round(k*kinv) flips rare: ~1.4e-3 output rel err (gate is 2e-2).
  - finalize: Pool partition_all_reduce -> amax; sk = amax/127 (ACT, bf16);
    kinv = 127/amax (DVE reciprocal; ACT reciprocal is banned for accuracy).
  - pass-2 is DMA-free: Pool multiplies the fp16 stash by kinv into an f32
    staging tile (walrus accepts Pool mult with full-tile output; max/stt on
    Pool are rejected), DVE magic-rounds it back into kq as bf16 ints. kq
    production is engine-paced, so the PE ramps right after finalize.
  - x quant: SP-ring loads (1MB halves), DVE abs-max reduces; the magic round
    runs as op1 (t = x*inv + MAGIC: DVE for tiles 0-3 which land during
    pass-1 while ACT streams k; ACT Copy(scale=inv, bias=MAGIC) for tiles
    4-15) and op2 (t - MAGIC -> bf16 ints, DVE, quarter tiles). xq is stored
    to DRAM in an mt-blocked [mt, dc, 128m, 128f] layout; ONE contiguous 1MB
    XBAR transpose-read per mt yields lhsT [128d, dc, 128m].
  - DVE max-accs are emitted interleaved with x tiles 0-3 so the 2-slot kt
    staging ring keeps releasing and the k DMA stream never stalls on DVE.
  - epilogue: one DVE scalar_tensor_tensor per [128, 512] PSUM tile
    (osb = (psum*sx)*sk, sk in bf16), out DMA on the ACT ring so the SP ring
    (x loads / xq writes / transposes) never queues behind output stores.
  - DMA instruction count per iteration: ~210 (vs ~700 in v6) — HWDGE issue
    serialization (~0.6us each) was a main v6 bottleneck.

The body is fully self-contained (no prologue): single-shot N=1 runs the same
instruction stream as each loop iteration, and the body is idempotent (kacc
memset + identical recompute each iteration), so the N-delta benchmark measures
the same program the harness runs once.
"""

import os
import sys
from contextlib import ExitStack

import numpy as np

if "/opt/trn_rl_repo" not in sys.path and os.path.isdir("/opt/trn_rl_repo"):
    sys.path.insert(0, "/opt/trn_rl_repo")

import concourse.bass as bass
import concourse.mybir as mybir
import concourse.tile as tile
from concourse import bacc, bass_isa

# Problem geometry (hardcoded per contract)
B, S, DIM, F_FULL = 4, 2048, 4096, 4096
M_FULL = B * S              # 8192 flattened rows
N_CORES = 8
M_SHARDS, F_SHARDS = 4, 2   # core c -> (mi, fi) = divmod(c, F_SHARDS)
M = M_FULL // M_SHARDS      # 2048 rows per core
F = F_FULL // F_SHARDS      # 2048 output cols per core
P = 128
DCH = DIM // P              # 32 contraction chunks
MT = M // P                 # 16 row tiles
FS = 512                    # matmul free dim (one PSUM bank of fp32)
FT = F // FS                # 4 output column strips
NSB = MT // 2               # 8 superblocks of 2 row tiles

MAGIC = float(np.float32(1.5 * 2**23))  # 12582912.0
INT8_MAX = 127.0

f32 = mybir.dt.float32
bf16 = mybir.dt.bfloat16
f16 = mybir.dt.float16
Alu = mybir.AluOpType
Act = mybir.ActivationFunctionType


def build_bass(niter: int = 1):
    """niter > 1 wraps the body in a hardware For loop — used only for
    benchmarking (kernel time = delta(wall) / delta(niter) cancels host I/O).
    The body is idempotent so the loop repeats the exact single-shot work."""
    nc = bacc.Bacc("TRN2", target_bir_lowering=False, enable_partition_id=False)

    x_in = nc.dram_tensor("x", [M, DIM], f32, kind="ExternalInput")
    k_in = nc.dram_tensor("kern", [DIM, F], f32, kind="ExternalInput")
    out = nc.dram_tensor("out", [M, F], f32, kind="ExternalOutput")

    with tile.TileContext(nc) as tc, ExitStack() as ctx:
        dram = ctx.enter_context(tc.tile_pool(name="dram", bufs=1, space="DRAM"))
        # quantized x ints, mt-blocked layout [mt, dc, m, 128f]: the whole mt
        # block [dc*m, 128] is contiguous, so ONE 1MB XBAR transpose read per
        # mt produces lhsT [128d, dc, m] (strided sub-2KB sources are slow).
        xq_dram = dram.tile([MT, DCH, P, P], bf16)

        persist = ctx.enter_context(tc.tile_pool(name="persist", bufs=1))
        # resident quantized kernel: 32 full tiles (walrus rejects sliced Pool
        # outputs, so each chunk is its own full-tile tensor). Doubles as the
        # fp16 stash between pass-1 and pass-2 (bitcast view; same 2B/elem).
        FH = F // 2
        kqt = [[persist.tile([P, FH], bf16, tag=f"kq{dc}_{h}",
                             name=f"kq{dc}_{h}") for h in range(2)]
               for dc in range(DCH)]
        kinvh = [persist.tile([P, FH], f32, tag=f"kinv{h}", name=f"kinv{h}")
                 for h in range(2)]
        skh = [persist.tile([P, FH], bf16, tag=f"sk{h}", name=f"sk{h}")
               for h in range(2)]
        kmaxh = [persist.tile([P, FH], f16, tag=f"kmax{h}", name=f"kmax{h}")
                 for h in range(2)]
        kminh = [persist.tile([P, FH], f16, tag=f"kmin{h}", name=f"kmin{h}")
                 for h in range(2)]
        sx_all = persist.tile([P, MT], f32)     # per-row scales, col mt

        xhp = ctx.enter_context(tc.tile_pool(name="xh", bufs=2))
        xsp = ctx.enter_context(tc.tile_pool(name="xs", bufs=4))
        xqp = ctx.enter_context(tc.tile_pool(name="xqo", bufs=2))
        ktp = ctx.enter_context(tc.tile_pool(name="kt", bufs=4))
        xtp = ctx.enter_context(tc.tile_pool(name="xt", bufs=2))
        psp = ctx.enter_context(tc.tile_pool(name="ps", bufs=8, space="PSUM"))
        osp = ctx.enter_context(tc.tile_pool(name="osb", bufs=2))

        def emit_pass1_k(dc, h):
            """Stream one half-width k chunk (0.5MB): dma (ACT ring for h=0,
            SWDGE for h=1) + fp16 stash cast (ACT). Half-chunks in a 4-deep
            ring keep the DMA->cast->free chain off the critical path."""
            kt = ktp.tile([P, F // 2], f32, tag="kt", name=f"p1_{dc}_{h}")
            eng = nc.sync if h == 0 else nc.gpsimd
            eng.dma_start(kt[:], k_in[dc * P:(dc + 1) * P,
                                      h * (F // 2):(h + 1) * (F // 2)])
            nc.scalar.activation(kqt[dc][h][:].bitcast(f16), kt[:], Act.Copy)

        def emit_accum(dc):
            """max/min-accumulate the fp16 stash (2-byte DVE fast path).
            amax derived from the stash is off by <=2^-11 relative — the
            resulting extra rounding flips stay well inside the 2e-2 gate."""
            for h in range(2):
                nc.vector.tensor_tensor(kmaxh[h][:], kqt[dc][h][:].bitcast(f16),
                                        kmaxh[h][:], Alu.max)
                nc.vector.tensor_tensor(kminh[h][:], kqt[dc][h][:].bitcast(f16),
                                        kminh[h][:], Alu.min)

        def emit_finalize():
            """Per half: amax = max(kmax, -kmin) -> f32; kinv = 127/amax;
            sk = amax/127 (bf16, epilogue-only)."""
            u16 = mybir.dt.uint16
            for h in range(2):
                nc.vector.tensor_scalar(kminh[h][:].bitcast(u16),
                                        kminh[h][:].bitcast(u16),
                                        0x8000, None, Alu.bitwise_xor)
                nc.vector.tensor_tensor(kmaxh[h][:], kminh[h][:], kmaxh[h][:],
                                        Alu.max)
                amaxf = ktp.tile([P, F // 2], f32, tag="kt", name=f"amaxf{h}")
                nc.vector.tensor_scalar(amaxf[:], kmaxh[h][:], 0.0, None,
                                        Alu.add)
                nc.gpsimd.partition_all_reduce(kinvh[h][:], amaxf[:], P,
                                               bass_isa.ReduceOp.max)
                nc.vector.tensor_scalar_max(kinvh[h][:], kinvh[h][:], 1e-6)
                nc.scalar.activation(skh[h][:], kinvh[h][:], Act.Copy,
                                     scale=1.0 / INT8_MAX)
                nc.vector.reciprocal(kinvh[h][:], kinvh[h][:])
                nc.vector.tensor_scalar_mul(kinvh[h][:], kinvh[h][:], INT8_MAX)

        def emit_pass2():
            """DMA-free quantize: kq = round(stash_fp16 * kinv) as bf16 ints.
            Mults alternate Pool/DVE so kq production outpaces mm consumption
            (Pool-only mults at 4us/chunk starved the PE for ~100us)."""
            for dc in range(DCH):
                for h in range(2):
                    kt2 = ktp.tile([P, F // 2], f32, tag="kt",
                                   name=f"p2_{dc}_{h}")
                    eng = nc.gpsimd if (2 * dc + h) % 2 == 0 else nc.vector
                    eng.tensor_tensor(kt2[:], kqt[dc][h][:].bitcast(f16),
                                      kinvh[h][:], Alu.mult)
                    if (2 * dc + h) % 2 == 0:
                        nc.vector.tensor_scalar(kqt[dc][h][:], kt2[:], MAGIC,
                                                -MAGIC, Alu.add, Alu.add)
                    else:
                        # ACT round: in-place +MAGIC (f32), then -MAGIC->bf16
                        nc.scalar.activation(kt2[:], kt2[:], Act.Copy,
                                             bias=MAGIC)
                        nc.scalar.activation(kqt[dc][h][:], kt2[:], Act.Copy,
                                             bias=-MAGIC)

        def emit_xquant(mt, act_round):
            rows = slice(mt * P, (mt + 1) * P)
            xh0 = xhp.tile([P, DIM // 2], f32, tag="xh", name=f"xh0_{mt}")
            xh1 = xhp.tile([P, DIM // 2], f32, tag="xh", name=f"xh1_{mt}")
            nc.sync.dma_start(xh0[:], x_in[rows, :DIM // 2])
            nc.sync.dma_start(xh1[:], x_in[rows, DIM // 2:])
            a0 = xsp.tile([P, 1], f32, tag="ax", name=f"a0_{mt}")
            a1 = xsp.tile([P, 1], f32, tag="ax", name=f"a1_{mt}")
            nc.vector.tensor_reduce(a0[:], xh0[:], axis=mybir.AxisListType.X,
                                    op=Alu.max, apply_absolute_value=True)
            nc.vector.tensor_reduce(a1[:], xh1[:], axis=mybir.AxisListType.X,
                                    op=Alu.max, apply_absolute_value=True)
            ax = xsp.tile([P, 1], f32, tag="ax", name=f"ax_{mt}")
            nc.vector.tensor_tensor(ax[:], a0[:], a1[:], Alu.max)
            # sx = max(ax, 1e-6)/127 in one op; inv = 1/sx = 127/amax
            nc.vector.tensor_scalar(sx_all[:, mt:mt + 1], ax[:], 1e-6,
                                    1.0 / INT8_MAX, Alu.max, Alu.mult)
            inv = xsp.tile([P, 1], f32, tag="ax", name=f"inv_{mt}")
            nc.vector.reciprocal(inv[:], sx_all[:, mt:mt + 1])
            for h, xh in ((0, xh0), (1, xh1)):
                # op1: t = x*inv + MAGIC (fp32, in place). ACT during the mm
                # phase (idle there), Pool during pass-1 (idle there; ACT is
                # busy with k casts and DVE with amax accumulation).
                if act_round:
                    nc.scalar.activation(xh[:], xh[:], Act.Copy,
                                         scale=inv[:, :1], bias=MAGIC)
                else:
                    nc.gpsimd.tensor_scalar(xh[:], xh[:], inv[:, :1], MAGIC,
                                            Alu.mult, Alu.add)
                # op2: quarters, t - MAGIC -> bf16 ints, then store
                for q in range(2):
                    qcols = slice(q * (DIM // 4), (q + 1) * (DIM // 4))
                    xqo = xqp.tile([P, DIM // 4], bf16, tag="xqo",
                                   name=f"xqo{h}{q}_{mt}")
                    nc.vector.tensor_scalar(xqo[:], xh[:, qcols], -MAGIC, None,
                                            Alu.add)
                    dc0 = h * (DCH // 2) + q * (DCH // 4)
                    nc.sync.dma_start(
                        xq_dram[mt, dc0:dc0 + DCH // 4]
                        .rearrange("dc m f -> m dc f"),
                        xqo[:].rearrange("m (dc f) -> m dc f", f=P))

        def emit_transpose(mt):
            xt = xtp.tile([P, DCH, P], bf16, tag="xqT", name=f"xt_{mt}")
            nc.sync.dma_start_transpose(
                xt[:], xq_dram[mt].rearrange("dc m f -> (dc m) f"))
            return xt

        def emit_superblock(sb, xts):
            for ml in range(2):
                mt = 2 * sb + ml
                mrows = slice(mt * P, (mt + 1) * P)
                psums = [psp.tile([P, FS], f32, tag="ps", name=f"ps{mt}_{i}")
                         for i in range(FT)]
                for dc in range(DCH):
                    lhsT = xts[ml][:, dc, :]
                    for fs in range(FT):
                        nc.tensor.matmul(
                            psums[fs][:], lhsT,
                            kqt[dc][fs // 2][:, (fs % 2) * FS:(fs % 2 + 1) * FS],
                            start=(dc == 0), stop=(dc == DCH - 1))
                for fs in range(FT):
                    osb = osp.tile([P, FS], f32, tag="osb",
                                   name=f"osb{mt}_{fs}")
                    # fused epilogue: osb = (psum * sx) * sk in one DVE op
                    nc.vector.scalar_tensor_tensor(
                        osb[:], psums[fs][:], sx_all[:, mt:mt + 1],
                        skh[fs // 2][:, (fs % 2) * FS:(fs % 2 + 1) * FS],
                        Alu.mult, Alu.mult)
                    nc.scalar.dma_start(out[mrows, fs * FS:(fs + 1) * FS],
                                        osb[:])

        def emit_body():
            for h in range(2):
                nc.vector.memset(kmaxh[h][:], float("-inf"))
                nc.vector.memset(kminh[h][:], float("inf"))
            # pass-1 k DMA+cast stream first (ring drains at DMA pace), then
            # x tiles 0-3 interleaved with the decoupled stash accumulation.
            for dc in range(DCH):
                emit_pass1_k(dc, 0)
                emit_pass1_k(dc, 1)
            for g in range(4):
                emit_xquant(g, act_round=False)
                for dc in range(g * 8, (g + 1) * 8):
                    emit_accum(dc)
            emit_finalize()
            emit_pass2()
            xts = [emit_transpose(0), emit_transpose(1)]
            for sb in range(NSB):
                nxts = None
                if sb < NSB - 1:
                    if 2 * sb + 4 < MT:
                        emit_xquant(2 * sb + 4, act_round=True)
                    if 2 * sb + 5 < MT:
                        emit_xquant(2 * sb + 5, act_round=True)
                    nxts = [emit_transpose(2 * sb + 2),
                            emit_transpose(2 * sb + 3)]
                emit_superblock(sb, xts)
                xts = nxts

        if niter > 1:
            with tc.For_i(0, niter, 1):
                emit_body()
        else:
            emit_body()

    nc.compile()
    return nc


_NC_CACHE = None


def _get_nc():
    global _NC_CACHE
    if _NC_CACHE is None:
        _NC_CACHE = build_bass()
    return _NC_CACHE


def make_in_maps(inputs: np.ndarray, kernel: np.ndarray):
    x = np.ascontiguousarray(np.asarray(inputs, np.float32).reshape(M_FULL, DIM))
    w = np.asarray(kernel, np.float32)
    in_maps = []
    for c in range(N_CORES):
        mi, fi = divmod(c, F_SHARDS)
        in_maps.append({
            "x": np.ascontiguousarray(x[mi * M:(mi + 1) * M]),
            "kern": np.ascontiguousarray(w[:, fi * F:(fi + 1) * F]),
        })
    return in_maps


def assemble_out(shards):
    out = np.empty((M_FULL, F_FULL), np.float32)
    for c in range(N_CORES):
        mi, fi = divmod(c, F_SHARDS)
        out[mi * M:(mi + 1) * M, fi * F:(fi + 1) * F] = shards[c]
    return out.reshape(B, S, F_FULL)


def kernel(inputs: np.ndarray, kernel: np.ndarray, _trace: bool = False):
    from concourse.bass_utils import run_bass_kernel_spmd

    nc = _get_nc()
    res = run_bass_kernel_spmd(nc, make_in_maps(inputs, kernel),
                               core_ids=list(range(N_CORES)), trace=_trace)
    out = assemble_out([r["out"] for r in res.results])
    if _trace:
        return out, res
    return out


# revision 38
# speedup vs baseline: 1.1624x; 1.0339x over previous
"""Trainium2 Bass kernel for quantized DenseGeneral (AQT-style int8 fake-quant matmul).

Math (must match the jax reference):
  xq = round(x / sx) * sx    sx = max(amax_row(|x|), 1e-6)/127     (per-row of x)
  kq = round(k / sk) * sk    sk = max(amax_col(|k|), 1e-6)/127     (per-col of k)
  out = xq @ kq  =  (sx (x) sk) * (x_int @ k_int)

x_int/k_int are integers in [-127, 127] — exactly representable in bf16, so the
matmul runs on the PE array in bf16 with fp32 PSUM accumulation and is exact.
round() uses the fp32 magic-number trick (t + 1.5*2^23 - 1.5*2^23), matching
jnp.round's round-half-to-even.

Sharding over 8 cores: 4-way on flattened rows, 2-way on output columns F.
Per core: x [2048, 4096], k [4096, 2048] -> out [2048, 2048].

v7 — single k read with fp16 stash + engine rebalance + big DMAs:
  - pass-1 streams k once (32 x 1MB on the ACT HWDGE ring). Per chunk: ACT
    casts the f32 chunk to fp16 and stashes it into the resident kq buffer
    (bitcast view), ACT takes |kt| in place, DVE max-accumulates the exact
    f32 amax into kacc. fp16 (10 mantissa bits) makes round(fp16(k)*kinv)
    vs round(k*kinv) flips rare: ~1.4e-3 output rel err (gate is 2e-2).
  - finalize: Pool partition_all_reduce -> amax; sk = amax/127 (ACT, bf16);
    kinv = 127/amax (DVE reciprocal; ACT reciprocal is banned for accuracy).
  - pass-2 is DMA-free: Pool multiplies the fp16 stash by kinv into an f32
    staging tile (walrus accepts Pool mult with full-tile output; max/stt on
    Pool are rejected), DVE magic-rounds it back into kq as bf16 ints. kq
    production is engine-paced, so the PE ramps right after finalize.
  - x quant: SP-ring loads (1MB halves), DVE abs-max reduces; the magic round
    runs as op1 (t = x*inv + MAGIC: DVE for tiles 0-3 which land during
    pass-1 while ACT streams k; ACT Copy(scale=inv, bias=MAGIC) for tiles
    4-15) and op2 (t - MAGIC -> bf16 ints, DVE, quarter tiles). xq is stored
    to DRAM in an mt-blocked [mt, dc, 128m, 128f] layout; ONE contiguous 1MB
    XBAR transpose-read per mt yields lhsT [128d, dc, 128m].
  - DVE max-accs are emitted interleaved with x tiles 0-3 so the 2-slot kt
    staging ring keeps releasing and the k DMA stream never stalls on DVE.
  - epilogue: one DVE scalar_tensor_tensor per [128, 512] PSUM tile
    (osb = (psum*sx)*sk, sk in bf16), out DMA on the ACT ring so the SP ring
    (x loads / xq writes / transposes) never queues behind output stores.
  - DMA instruction count per iteration: ~210 (vs ~700 in v6) — HWDGE issue
    serialization (~0.6us each) was a main v6 bottleneck.

The body is fully self-contained (no prologue): single-shot N=1 runs the same
instruction stream as each loop iteration, and the body is idempotent (kacc
memset + identical recompute each iteration), so the N-delta benchmark measures
the same program the harness runs once.
"""

import os
import sys
from contextlib import ExitStack

import numpy as np

if "/opt/trn_rl_repo" not in sys.path and os.path.isdir("/opt/trn_rl_repo"):
    sys.path.insert(0, "/opt/trn_rl_repo")

import concourse.bass as bass
import concourse.mybir as mybir
import concourse.tile as tile
from concourse import bacc, bass_isa

# Problem geometry (hardcoded per contract)
B, S, DIM, F_FULL = 4, 2048, 4096, 4096
M_FULL = B * S              # 8192 flattened rows
N_CORES = 8
M_SHARDS, F_SHARDS = 4, 2   # core c -> (mi, fi) = divmod(c, F_SHARDS)
M = M_FULL // M_SHARDS      # 2048 rows per core
F = F_FULL // F_SHARDS      # 2048 output cols per core
P = 128
DCH = DIM // P              # 32 contraction chunks
MT = M // P                 # 16 row tiles
FS = 512                    # matmul free dim (one PSUM bank of fp32)
FT = F // FS                # 4 output column strips
NSB = MT // 2               # 8 superblocks of 2 row tiles

MAGIC = float(np.float32(1.5 * 2**23))  # 12582912.0
INT8_MAX = 127.0

f32 = mybir.dt.float32
bf16 = mybir.dt.bfloat16
f16 = mybir.dt.float16
Alu = mybir.AluOpType
Act = mybir.ActivationFunctionType


def build_bass(niter: int = 1):
    """niter > 1 wraps the body in a hardware For loop — used only for
    benchmarking (kernel time = delta(wall) / delta(niter) cancels host I/O).
    The body is idempotent so the loop repeats the exact single-shot work."""
    nc = bacc.Bacc("TRN2", target_bir_lowering=False, enable_partition_id=False)

    x_in = nc.dram_tensor("x", [M, DIM], f32, kind="ExternalInput")
    k_in = nc.dram_tensor("kern", [DIM, F], f32, kind="ExternalInput")
    out = nc.dram_tensor("out", [M, F], f32, kind="ExternalOutput")

    with tile.TileContext(nc) as tc, ExitStack() as ctx:
        dram = ctx.enter_context(tc.tile_pool(name="dram", bufs=1, space="DRAM"))
        # quantized x ints, mt-blocked layout [mt, dc, m, 128f]: the whole mt
        # block [dc*m, 128] is contiguous, so ONE 1MB XBAR transpose read per
        # mt produces lhsT [128d, dc, m] (strided sub-2KB sources are slow).
        xq_dram = dram.tile([MT, DCH, P, P], bf16)

        persist = ctx.enter_context(tc.tile_pool(name="persist", bufs=1))
        # resident quantized kernel: 32 full tiles (walrus rejects sliced Pool
        # outputs, so each chunk is its own full-tile tensor). Doubles as the
        # fp16 stash between pass-1 and pass-2 (bitcast view; same 2B/elem).
        FH = F // 2
        kqt = [[persist.tile([P, FH], bf16, tag=f"kq{dc}_{h}",
                             name=f"kq{dc}_{h}") for h in range(2)]
               for dc in range(DCH)]
        kinvh = [persist.tile([P, FH], f32, tag=f"kinv{h}", name=f"kinv{h}")
                 for h in range(2)]
        skh = [persist.tile([P, FH], bf16, tag=f"sk{h}", name=f"sk{h}")
               for h in range(2)]
        kmaxh = [persist.tile([P, FH], f16, tag=f"kmax{h}", name=f"kmax{h}")
                 for h in range(2)]
        sx_all = persist.tile([P, MT], f32)     # per-row scales, col mt

        xhp = ctx.enter_context(tc.tile_pool(name="xh", bufs=2))
        xsp = ctx.enter_context(tc.tile_pool(name="xs", bufs=4))
        xqp = ctx.enter_context(tc.tile_pool(name="xqo", bufs=2))
        ktp = ctx.enter_context(tc.tile_pool(name="kt", bufs=4))
        xtp = ctx.enter_context(tc.tile_pool(name="xt", bufs=2))
        psp = ctx.enter_context(tc.tile_pool(name="ps", bufs=8, space="PSUM"))
        osp = ctx.enter_context(tc.tile_pool(name="osb", bufs=2))

        def emit_pass1_k(dc, h):
            """Stream one half-width k chunk (0.5MB): dma (ACT ring for h=0,
            SWDGE for h=1) + fp16 stash cast (ACT). Half-chunks in a 4-deep
            ring keep the DMA->cast->free chain off the critical path."""
            kt = ktp.tile([P, F // 2], f32, tag="kt", name=f"p1_{dc}_{h}")
            eng = nc.sync if h == 0 else nc.gpsimd
            eng.dma_start(kt[:], k_in[dc * P:(dc + 1) * P,
                                      h * (F // 2):(h + 1) * (F // 2)])
            nc.scalar.activation(kqt[dc][h][:].bitcast(f16), kt[:], Act.Copy)

        def emit_accum(dc):
            """|stash| max-accumulate: clear the f16 sign bit into a small
            temp (u16 tensor_scalar, 4x 2-byte path), then f16 max into the
            single accumulator. amax derived from the stash is off by
            <=2^-11 relative; the flips stay well inside the 2e-2 gate."""
            u16 = mybir.dt.uint16
            for h in range(2):
                kab = xsp.tile([P, F // 2], f16, tag="kab", bufs=2,
                               name=f"kab_{dc}_{h}")
                nc.vector.tensor_scalar(kab[:].bitcast(u16),
                                        kqt[dc][h][:].bitcast(u16),
                                        0x7FFF, None, Alu.bitwise_and)
                nc.vector.tensor_tensor(kmaxh[h][:], kab[:], kmaxh[h][:],
                                        Alu.max)

        def emit_finalize():
            """Per half: kmax already holds |stash| amax -> f32;
            kinv = 127/amax; sk = amax/127 (bf16, epilogue-only)."""
            for h in range(2):
                amaxf = ktp.tile([P, F // 2], f32, tag="kt", name=f"amaxf{h}")
                nc.vector.tensor_scalar(amaxf[:], kmaxh[h][:], 0.0, None,
                                        Alu.add)
                nc.gpsimd.partition_all_reduce(kinvh[h][:], amaxf[:], P,
                                               bass_isa.ReduceOp.max)
                nc.vector.tensor_scalar_max(kinvh[h][:], kinvh[h][:], 1e-6)
                nc.scalar.activation(skh[h][:], kinvh[h][:], Act.Copy,
                                     scale=1.0 / INT8_MAX)
                nc.vector.reciprocal(kinvh[h][:], kinvh[h][:])
                nc.vector.tensor_scalar_mul(kinvh[h][:], kinvh[h][:], INT8_MAX)

        def emit_pass2():
            """DMA-free quantize: kq = round(stash_fp16 * kinv) as bf16 ints.
            Mults alternate Pool/DVE so kq production outpaces mm consumption
            (Pool-only mults at 4us/chunk starved the PE for ~100us)."""
            for dc in range(DCH):
                for h in range(2):
                    kt2 = ktp.tile([P, F // 2], f32, tag="kt",
                                   name=f"p2_{dc}_{h}")
                    eng = nc.gpsimd if (2 * dc + h) % 2 == 0 else nc.vector
                    eng.tensor_tensor(kt2[:], kqt[dc][h][:].bitcast(f16),
                                      kinvh[h][:], Alu.mult)
                    if (2 * dc + h) % 2 == 0:
                        nc.vector.tensor_scalar(kqt[dc][h][:], kt2[:], MAGIC,
                                                -MAGIC, Alu.add, Alu.add)
                    else:
                        # ACT round: in-place +MAGIC (f32), then -MAGIC->bf16
                        nc.scalar.activation(kt2[:], kt2[:], Act.Copy,
                                             bias=MAGIC)
                        nc.scalar.activation(kqt[dc][h][:], kt2[:], Act.Copy,
                                             bias=-MAGIC)

        def emit_xquant(mt, act_round):
            rows = slice(mt * P, (mt + 1) * P)
            xh0 = xhp.tile([P, DIM // 2], f32, tag="xh", name=f"xh0_{mt}")
            xh1 = xhp.tile([P, DIM // 2], f32, tag="xh", name=f"xh1_{mt}")
            nc.sync.dma_start(xh0[:], x_in[rows, :DIM // 2])
            nc.sync.dma_start(xh1[:], x_in[rows, DIM // 2:])
            a0 = xsp.tile([P, 1], f32, tag="ax", name=f"a0_{mt}")
            a1 = xsp.tile([P, 1], f32, tag="ax", name=f"a1_{mt}")
            nc.vector.tensor_reduce(a0[:], xh0[:], axis=mybir.AxisListType.X,
                                    op=Alu.max, apply_absolute_value=True)
            nc.vector.tensor_reduce(a1[:], xh1[:], axis=mybir.AxisListType.X,
                                    op=Alu.max, apply_absolute_value=True)
            ax = xsp.tile([P, 1], f32, tag="ax", name=f"ax_{mt}")
            nc.vector.tensor_tensor(ax[:], a0[:], a1[:], Alu.max)
            # sx = max(ax, 1e-6)/127 in one op; inv = 1/sx = 127/amax
            nc.vector.tensor_scalar(sx_all[:, mt:mt + 1], ax[:], 1e-6,
                                    1.0 / INT8_MAX, Alu.max, Alu.mult)
            inv = xsp.tile([P, 1], f32, tag="ax", name=f"inv_{mt}")
            nc.vector.reciprocal(inv[:], sx_all[:, mt:mt + 1])
            for h, xh in ((0, xh0), (1, xh1)):
                # op1: t = x*inv + MAGIC (fp32, in place). ACT during the mm
                # phase (idle there), Pool during pass-1 (idle there; ACT is
                # busy with k casts and DVE with amax accumulation).
                if act_round:
                    nc.scalar.activation(xh[:], xh[:], Act.Copy,
                                         scale=inv[:, :1], bias=MAGIC)
                else:
                    nc.gpsimd.tensor_scalar(xh[:], xh[:], inv[:, :1], MAGIC,
                                            Alu.mult, Alu.add)
                # op2: quarters, t - MAGIC -> bf16 ints, then store
                for q in range(2):
                    qcols = slice(q * (DIM // 4), (q + 1) * (DIM // 4))
                    xqo = xqp.tile([P, DIM // 4], bf16, tag="xqo",
                                   name=f"xqo{h}{q}_{mt}")
                    nc.vector.tensor_scalar(xqo[:], xh[:, qcols], -MAGIC, None,
                                            Alu.add)
                    dc0 = h * (DCH // 2) + q * (DCH // 4)
                    nc.sync.dma_start(
                        xq_dram[mt, dc0:dc0 + DCH // 4]
                        .rearrange("dc m f -> m dc f"),
                        xqo[:].rearrange("m (dc f) -> m dc f", f=P))

        def emit_transpose(mt):
            xt = xtp.tile([P, DCH, P], bf16, tag="xqT", name=f"xt_{mt}")
            nc.sync.dma_start_transpose(
                xt[:], xq_dram[mt].rearrange("dc m f -> (dc m) f"))
            return xt

        def emit_superblock(sb, xts):
            for ml in range(2):
                mt = 2 * sb + ml
                mrows = slice(mt * P, (mt + 1) * P)
                psums = [psp.tile([P, FS], f32, tag="ps", name=f"ps{mt}_{i}")
                         for i in range(FT)]
                for dc in range(DCH):
                    lhsT = xts[ml][:, dc, :]
                    for fs in range(FT):
                        nc.tensor.matmul(
                            psums[fs][:], lhsT,
                            kqt[dc][fs // 2][:, (fs % 2) * FS:(fs % 2 + 1) * FS],
                            start=(dc == 0), stop=(dc == DCH - 1))
                for fs in range(FT):
                    osb = osp.tile([P, FS], f32, tag="osb",
                                   name=f"osb{mt}_{fs}")
                    # fused epilogue: osb = (psum * sx) * sk in one DVE op
                    nc.vector.scalar_tensor_tensor(
                        osb[:], psums[fs][:], sx_all[:, mt:mt + 1],
                        skh[fs // 2][:, (fs % 2) * FS:(fs % 2 + 1) * FS],
                        Alu.mult, Alu.mult)
                    nc.scalar.dma_start(out[mrows, fs * FS:(fs + 1) * FS],
                                        osb[:])

        def emit_body():
            for h in range(2):
                nc.vector.memset(kmaxh[h][:], 0.0)
            # pass-1 k DMA+cast stream first (ring drains at DMA pace), then
            # x tiles 0-3 interleaved with the decoupled stash accumulation.
            for dc in range(DCH):
                emit_pass1_k(dc, 0)
                emit_pass1_k(dc, 1)
            for g in range(4):
                emit_xquant(g, act_round=False)
                for dc in range(g * 8, (g + 1) * 8):
                    emit_accum(dc)
            emit_finalize()
            emit_pass2()
            xts = [emit_transpose(0), emit_transpose(1)]
            for sb in range(NSB):
                nxts = None
                if sb < NSB - 1:
                    if 2 * sb + 4 < MT:
                        emit_xquant(2 * sb + 4, act_round=True)
                    if 2 * sb + 5 < MT:
                        emit_xquant(2 * sb + 5, act_round=True)
                    nxts = [emit_transpose(2 * sb + 2),
                            emit_transpose(2 * sb + 3)]
                emit_superblock(sb, xts)
                xts = nxts

        if niter > 1:
            with tc.For_i(0, niter, 1):
                emit_body()
        else:
            emit_body()

    nc.compile()
    return nc


_NC_CACHE = None


def _get_nc():
    global _NC_CACHE
    if _NC_CACHE is None:
        _NC_CACHE = build_bass()
    return _NC_CACHE


def make_in_maps(inputs: np.ndarray, kernel: np.ndarray):
    x = np.ascontiguousarray(np.asarray(inputs, np.float32).reshape(M_FULL, DIM))
    w = np.asarray(kernel, np.float32)
    in_maps = []
    for c in range(N_CORES):
        mi, fi = divmod(c, F_SHARDS)
        in_maps.append({
            "x": np.ascontiguousarray(x[mi * M:(mi + 1) * M]),
            "kern": np.ascontiguousarray(w[:, fi * F:(fi + 1) * F]),
        })
    return in_maps


def assemble_out(shards):
    out = np.empty((M_FULL, F_FULL), np.float32)
    for c in range(N_CORES):
        mi, fi = divmod(c, F_SHARDS)
        out[mi * M:(mi + 1) * M, fi * F:(fi + 1) * F] = shards[c]
    return out.reshape(B, S, F_FULL)


def kernel(inputs: np.ndarray, kernel: np.ndarray, _trace: bool = False):
    from concourse.bass_utils import run_bass_kernel_spmd

    nc = _get_nc()
    res = run_bass_kernel_spmd(nc, make_in_maps(inputs, kernel),
                               core_ids=list(range(N_CORES)), trace=_trace)
    out = assemble_out([r["out"] for r in res.results])
    if _trace:
        return out, res
    return out
